# revision 7
# baseline (speedup 1.0000x reference)
"""Trainium2 Bass kernel for nn_ExpertGroup (moe_routing).

Reference computation (B=2, S=1024, E=768, NE=8, H=3072, A=192):
    shared = silu(x @ up_w.T)                     # [B,S,H]
    pre    = shared @ adapt_w.T                   # [B,S,A]
    for i in 0..7:
        h  = LN(pre @ adapter_w[i].T) * g[i] + b[i]
        o  = (h @ expert_proj_w.T) @ output_proj_w.T
        combined = where(mask_i, o, combined)     # overwrite: later experts win
    out = shared + 0.1 * combined

Algebraic restructure: the mask-overwrite selects, per token, the LAST
expert i with expert_weights > 0 (or zero if none).  expert_proj/
output_proj are shared across experts and linear, so selection commutes
with them; LN is per-token so selection commutes with LN too.  Per token
only ONE expert's tiny A->A adapter matmul is needed:
    sel  = sum_i onehot_i * (pre @ adapter_w[i].T)   # one-hot from host
    g    = LN(sel) * (0.1*gamma[e(t)]) + (0.1*beta[e(t)])   # 0 if unrouted
    out  = shared + (g @ expert_proj_w.T) @ output_proj_w.T
This removes the 8x duplication of the big GEMMs: ~53 GFLOP total.

Sharding: data-parallel over the 2048 tokens, 256 per core, weights
replicated (streamed from HBM once per core).  Device layout is
feature-major ("transposed", [feature, token]) except the adapter/LN
stage which is token-major so LN reduces along the free axis.
"""

import sys

if "/opt/trn_rl_repo" not in sys.path:
    sys.path.insert(0, "/opt/trn_rl_repo")

import numpy as np
import ml_dtypes

import concourse.bass as bass
import concourse.bacc as bacc
import concourse.mybir as mybir
import concourse.tile as tile
from concourse.masks import make_identity
from concourse.vector_clock import ScopedClock
from concourse.bass_utils import run_bass_kernel_spmd

BF16 = ml_dtypes.bfloat16

B, S, E, NE = 2, 1024, 768, 8
H = 4 * E            # 3072
A = H // 16          # 192
LN_EPS = 1e-5
N_CORES = 8
T = (B * S) // N_CORES   # 256 tokens per core
P = 128

KE = E // P          # 6  k-tiles over E
MH = H // P          # 24 m-tiles over H
KH = H // P          # 24 k-tiles over H
TH = T // P          # 2  token halves

# Matmul dtypes (tunable): float32r streams 1 row/cycle for free-dim>=256
# (same as bf16) while keeping fp32 storage.  The expert path is ~1e-5 of
# the output magnitude, so bf16 there is far below the noise floor.
DT_A = mybir.dt.float32r     # up-proj (dominates output accuracy)
DT_REST = mybir.dt.bfloat16  # adapt/adapter/expert_proj/output_proj path
F32 = mybir.dt.float32

NP_DT = {mybir.dt.float32r: np.float32,
         mybir.dt.float32: np.float32,
         mybir.dt.bfloat16: BF16}


class SplitDrainTileContext(tile.TileContext):
    """TileContext whose exit drain splits its sem waits one-per-Drain:
    walrus's CTRL lowering in this container accepts only a single sync
    wait on a Drain instruction."""

    def _drain_and_barrier(self, tick_clock, wait_clock):
        nc = self.nc
        drain_inst = nc.sync.drain()
        wait_clock.add_sem_waits(
            drain_inst.ins, ScopedClock({None: tick_clock.global_clock})
        )
        si = drain_inst.ins.sync_info
        waits = list(si.on_wait) if si is not None else []
        if len(waits) > 1:
            drain_inst.ins.sync_info = mybir.SyncInfo(
                on_wait=waits[:1], on_update=list(si.on_update)
            )
            for k in range(1, len(waits)):
                extra = nc.sync.drain()
                extra.ins.sync_info = mybir.SyncInfo(
                    on_wait=waits[k:k + 1], on_update=[]
                )
        nc.all_engine_barrier()
        popped = nc._tile_sem_poison_stack.pop()
        assert popped is self._sem_poison
        nc.clear_and_free_semaphores(list(self.sems.allocated().values()))
        nc.all_engine_barrier()


def _build_program():
    nc = bacc.Bacc()

    # ---- DRAM I/O (per core) ----
    xT = nc.dram_tensor("xT", [E, T], DT_A, kind="ExternalInput")
    up_wT = nc.dram_tensor("up_wT", [E, H], DT_A, kind="ExternalInput")
    adapt_wT = nc.dram_tensor("adapt_wT", [H, A], DT_REST, kind="ExternalInput")
    awTcat = nc.dram_tensor("awTcat", [2 * P, NE * A], DT_REST, kind="ExternalInput")
    oh = nc.dram_tensor("oh", [T, NE], F32, kind="ExternalInput")
    gam = nc.dram_tensor("gam", [T, A], F32, kind="ExternalInput")
    bet = nc.dram_tensor("bet", [T, A], F32, kind="ExternalInput")
    ep_wT = nc.dram_tensor("ep_wT", [2 * P, H], DT_REST, kind="ExternalInput")
    op_wT = nc.dram_tensor("op_wT", [H, H], DT_REST, kind="ExternalInput")
    out = nc.dram_tensor("out", [H, T], F32, kind="ExternalOutput")

    with tile.TileContext(nc) as tc:
        with (
            tc.tile_pool(name="const", bufs=1) as const_pool,
            tc.tile_pool(name="xw", bufs=1) as x_pool,
            tc.tile_pool(name="upw", bufs=3) as up_pool,
            tc.tile_pool(name="shared", bufs=1) as shared_pool,
            tc.tile_pool(name="small_w", bufs=1) as smallw_pool,
            tc.tile_pool(name="adapter", bufs=1) as ad_pool,
            tc.tile_pool(name="tact", bufs=1) as tact_pool,
            tc.tile_pool(name="opw", bufs=4) as op_pool,
            tc.tile_pool(name="outs", bufs=4) as out_pool,
            tc.tile_pool(name="psbig", bufs=4, space="PSUM") as psA,
            tc.tile_pool(name="pssmall", bufs=4, space="PSUM") as psS,
        ):
            # ---------- load shared inputs ----------
            x_sb = x_pool.tile([P, KE, T], DT_A)
            nc.sync.dma_start(out=x_sb[:], in_=xT.rearrange("(k p) t -> p k t", p=P))

            adapt_sb = smallw_pool.tile([P, KH, A], DT_REST)
            nc.sync.dma_start(
                out=adapt_sb[:], in_=adapt_wT.rearrange("(k p) a -> p k a", p=P)
            )
            aw_sb = ad_pool.tile([P, 2, NE * A], DT_REST)
            nc.sync.dma_start(
                out=aw_sb[:], in_=awTcat.rearrange("(k p) n -> p k n", p=P)
            )
            ep_sb = smallw_pool.tile([P, 2, H], DT_REST)
            nc.sync.dma_start(
                out=ep_sb[:], in_=ep_wT.rearrange("(k p) h -> p k h", p=P)
            )
            oh_sb = const_pool.tile([P, TH, NE], F32)
            nc.sync.dma_start(out=oh_sb[:], in_=oh.rearrange("(n p) i -> p n i", p=P))
            gam_sb = const_pool.tile([P, TH, A], F32)
            nc.sync.dma_start(out=gam_sb[:], in_=gam.rearrange("(n p) c -> p n c", p=P))
            bet_sb = const_pool.tile([P, TH, A], F32)
            nc.sync.dma_start(out=bet_sb[:], in_=bet.rearrange("(n p) c -> p n c", p=P))
            ident = const_pool.tile([P, P], F32)
            make_identity(nc, ident)
            eps_tile = const_pool.tile([P, 1], F32)
            nc.any.memset(eps_tile[:], float(LN_EPS))

            up_r = up_wT.rearrange("(k p) h -> p k h", p=P)
            op_r = op_wT.rearrange("(k p) g -> p k g", p=P)

            # ---------- Stage A: sharedT = silu(up_wT.T @ xT) ----------
            shared_f32 = []   # 24 tiles [128, T] fp32 (residual + accuracy)
            shared_mm = []    # bf16 copies for stage B rhs
            for m in range(MH):
                up_strip = up_pool.tile([P, KE, P], DT_A, tag="up_strip")
                nc.sync.dma_start(
                    out=up_strip[:], in_=up_r[:, :, m * P:(m + 1) * P]
                )
                ps = psA.tile([P, T], F32, tag="ps")
                for k in range(KE):
                    nc.tensor.matmul(
                        ps[:], up_strip[:, k, :], x_sb[:, k, :],
                        start=(k == 0), stop=(k == KE - 1),
                    )
                sh = shared_pool.tile([P, T], F32, tag=f"sh{m}")
                nc.scalar.activation(sh[:], ps[:], mybir.ActivationFunctionType.Silu)
                shb = shared_pool.tile([P, T], DT_REST, tag=f"shb{m}")
                nc.vector.tensor_copy(shb[:], sh[:])
                shared_f32.append(sh)
                shared_mm.append(shb)

            # ---------- Stage B: preT = adapt_wT.T @ sharedT  [A, T] ----------
            # A=192 -> two partition groups (128 + 64)
            pre_mm = []   # [ [128,T], [64,T] ] in DT_REST
            for g in range(2):
                gp = P if g == 0 else A - P      # 128, 64
                ps = psA.tile([P, T], F32, tag="ps")
                for k in range(KH):
                    nc.tensor.matmul(
                        ps[:gp, :],
                        adapt_sb[:, k, g * P:g * P + gp],
                        shared_mm[k][:],
                        start=(k == 0), stop=(k == KH - 1),
                    )
                pb = shared_pool.tile([P, T], DT_REST, tag=f"pre{g}")
                nc.scalar.activation(
                    pb[:gp, :], ps[:gp, :], mybir.ActivationFunctionType.Copy
                )
                pre_mm.append(pb)

            # ---------- Stage D/E: adapter + select + LayerNorm (token-major) ----------
            # gT: feature-major normalized output [A, T] in DT_REST
            gT0 = shared_pool.tile([P, T], DT_REST, tag="gT0")   # c 0:128
            gT1 = shared_pool.tile([P, T], DT_REST, tag="gT1")   # c 128:192 (64 rows)
            for th in range(TH):
                tsl = slice(th * P, (th + 1) * P)
                hsel = shared_pool.tile([P, A], F32, tag="hsel")
                for i in range(NE):
                    ps = psS.tile([P, A], F32, tag="pss")
                    nc.tensor.matmul(
                        ps[:], pre_mm[0][:, tsl], aw_sb[:, 0, i * A:(i + 1) * A],
                        start=True, stop=False,
                    )
                    nc.tensor.matmul(
                        ps[:], pre_mm[1][:64, tsl], aw_sb[:64, 1, i * A:(i + 1) * A],
                        start=False, stop=True,
                    )
                    if i == 0:
                        nc.scalar.activation(
                            hsel[:], ps[:], mybir.ActivationFunctionType.Copy,
                            scale=oh_sb[:, th, i:i + 1],
                        )
                    else:
                        hm = shared_pool.tile([P, A], F32, tag="hm")
                        nc.scalar.activation(
                            hm[:], ps[:], mybir.ActivationFunctionType.Copy,
                            scale=oh_sb[:, th, i:i + 1],
                        )
                        nc.vector.tensor_add(hsel[:], hsel[:], hm[:])

                # LayerNorm over free axis (192), biased var, eps inside sqrt
                s1 = shared_pool.tile([P, 1], F32, tag="s1")
                nc.vector.reduce_sum(s1[:], hsel[:], axis=mybir.AxisListType.X)
                hsq = shared_pool.tile([P, A], F32, tag="hsq")
                s2 = shared_pool.tile([P, 1], F32, tag="s2")
                nc.scalar.activation(
                    hsq[:], hsel[:], mybir.ActivationFunctionType.Square,
                    accum_out=s2[:],
                )
                t1 = shared_pool.tile([P, 1], F32, tag="t1")
                nc.vector.tensor_mul(t1[:], s1[:], s1[:])
                nc.vector.tensor_scalar_mul(t1[:], t1[:], 1.0 / A)
                nc.vector.tensor_sub(t1[:], s2[:], t1[:])        # 192*var
                std = shared_pool.tile([P, 1], F32, tag="std")
                nc.scalar.activation(
                    std[:], t1[:], mybir.ActivationFunctionType.Sqrt,
                    scale=1.0 / A, bias=eps_tile[:],
                )
                rstd = shared_pool.tile([P, 1], F32, tag="rstd")
                nc.vector.reciprocal(rstd[:], std[:])
                nm = shared_pool.tile([P, 1], F32, tag="nm")
                nc.vector.tensor_mul(nm[:], s1[:], rstd[:])
                nc.vector.tensor_scalar_mul(nm[:], nm[:], -1.0 / A)
                gtok = shared_pool.tile([P, A], F32, tag="gtok")
                nc.scalar.activation(
                    gtok[:], hsel[:], mybir.ActivationFunctionType.Identity,
                    scale=rstd[:], bias=nm[:],
                )
                nc.vector.tensor_mul(gtok[:], gtok[:], gam_sb[:, th, :])
                nc.vector.tensor_add(gtok[:], gtok[:], bet_sb[:, th, :])

                # transpose to feature-major: [128t, 192c] -> [c, 128t]
                pst = psS.tile([P, P], F32, tag="pss")
                nc.tensor.transpose(pst[:], gtok[:, 0:P], ident[:])
                nc.scalar.activation(
                    gT0[:, tsl], pst[:], mybir.ActivationFunctionType.Copy
                )
                pst2 = psS.tile([P, P], F32, tag="pss")
                nc.tensor.transpose(pst2[:64, :], gtok[:, P:A], ident[:])
                nc.scalar.activation(
                    gT1[:64, tsl], pst2[:64, :], mybir.ActivationFunctionType.Copy
                )

            # ---------- Stage F: t_actT = ep_wT.T @ gT  [H, T] ----------
            tact_mm = []
            for m in range(MH):
                ps = psA.tile([P, T], F32, tag="ps")
                nc.tensor.matmul(
                    ps[:], ep_sb[:, 0, m * P:(m + 1) * P], gT0[:],
                    start=True, stop=False,
                )
                nc.tensor.matmul(
                    ps[:], ep_sb[:64, 1, m * P:(m + 1) * P], gT1[:64, :],
                    start=False, stop=True,
                )
                ta = tact_pool.tile([P, T], DT_REST, tag=f"ta{m}")
                nc.scalar.activation(
                    ta[:], ps[:], mybir.ActivationFunctionType.Copy
                )
                tact_mm.append(ta)

            # ---------- Stage G: out = sharedT + op_wT.T @ t_actT ----------
            for m in range(MH):
                op_strip = op_pool.tile([P, KH, P], DT_REST, tag="op_strip")
                nc.sync.dma_start(
                    out=op_strip[:], in_=op_r[:, :, m * P:(m + 1) * P]
                )
                ps = psA.tile([P, T], F32, tag="ps")
                for k in range(KH):
                    nc.tensor.matmul(
                        ps[:], op_strip[:, k, :], tact_mm[k][:],
                        start=(k == 0), stop=(k == KH - 1),
                    )
                ot = out_pool.tile([P, T], F32, tag="ot")
                nc.vector.tensor_add(ot[:], ps[:], shared_f32[m][:])
                nc.sync.dma_start(out=out[m * P:(m + 1) * P, :], in_=ot[:])

    nc.finalize()
    return nc


_NC_CACHE = None
LAST_RUN_S = None  # wall time of the last device dispatch (incl. RPC)


def _get_program():
    global _NC_CACHE
    if _NC_CACHE is None:
        _NC_CACHE = _build_program()
    return _NC_CACHE


def kernel(x, expert_weights, up_w, adapt_w, adapter_w, ln_gamma, ln_beta,
           expert_proj_w, output_proj_w):
    x = np.asarray(x, dtype=np.float32)
    expert_weights = np.asarray(expert_weights, dtype=np.float32)
    up_w = np.asarray(up_w, dtype=np.float32)
    adapt_w = np.asarray(adapt_w, dtype=np.float32)
    adapter_w = np.asarray(adapter_w, dtype=np.float32)
    ln_gamma = np.asarray(ln_gamma, dtype=np.float32)
    ln_beta = np.asarray(ln_beta, dtype=np.float32)
    expert_proj_w = np.asarray(expert_proj_w, dtype=np.float32)
    output_proj_w = np.asarray(output_proj_w, dtype=np.float32)

    NT = B * S  # 2048
    rest_np = NP_DT[DT_REST]

    # ---- routing (host): last expert with weight > 0, one-hot ----
    ew = expert_weights.reshape(NT, NE)
    pos = ew > 0
    idx = (NE - 1) - pos[:, ::-1].argmax(axis=1)       # last True (0 if none)
    valid = pos.any(axis=1)
    idx = np.where(valid, idx, 0)
    oh_full = np.zeros((NT, NE), np.float32)
    oh_full[np.arange(NT), idx] = valid.astype(np.float32)
    # fold the 0.1 output scale + unrouted-token zeroing into gamma/beta
    vmask = valid.astype(np.float32)[:, None]
    gam_full = (ln_gamma[idx] * 0.1 * vmask).astype(np.float32)
    bet_full = (ln_beta[idx] * 0.1 * vmask).astype(np.float32)

    # ---- weight prep (host, replicated across cores) ----
    xT_full = np.ascontiguousarray(x.reshape(NT, E).T)            # [E, NT] f32
    up_wT = np.ascontiguousarray(up_w.T)                          # [E, H] f32
    adapt_wT = np.ascontiguousarray(adapt_w.T).astype(rest_np)    # [H, A]
    awTcat = np.zeros((2 * P, NE * A), rest_np)                   # [256, 1536]
    awT = adapter_w.transpose(0, 2, 1)                            # [NE, A(in), A(out)]
    awTcat[:A, :] = np.ascontiguousarray(
        awT.transpose(1, 0, 2).reshape(A, NE * A)
    ).astype(rest_np)
    ep_wTp = np.zeros((2 * P, H), rest_np)
    ep_wTp[:A, :] = expert_proj_w.T.astype(rest_np)               # [A, H] padded
    op_wT = np.ascontiguousarray(output_proj_w.T).astype(rest_np)  # [H, H]

    in_maps = []
    for c in range(N_CORES):
        tsl = slice(c * T, (c + 1) * T)
        in_maps.append({
            "xT": np.ascontiguousarray(xT_full[:, tsl]),
            "up_wT": up_wT,
            "adapt_wT": adapt_wT,
            "awTcat": awTcat,
            "oh": np.ascontiguousarray(oh_full[tsl]),
            "gam": np.ascontiguousarray(gam_full[tsl]),
            "bet": np.ascontiguousarray(bet_full[tsl]),
            "ep_wT": ep_wTp,
            "op_wT": op_wT,
        })

    import time
    nc = _get_program()
    t0 = time.perf_counter()
    res = run_bass_kernel_spmd(nc, in_maps, list(range(N_CORES)))
    global LAST_RUN_S
    LAST_RUN_S = time.perf_counter() - t0

    outs = [res.results[c]["out"].T for c in range(N_CORES)]      # [T, H] each
    full = np.concatenate(outs, axis=0)                           # [NT, H]
    return np.ascontiguousarray(full.reshape(B, S, H)).astype(np.float32)


# revision 8
# speedup vs baseline: 1.2687x; 1.2687x over previous
"""Trainium2 Bass kernel for nn_ExpertGroup (moe_routing).

Reference computation (B=2, S=1024, E=768, NE=8, H=3072, A=192):
    shared = silu(x @ up_w.T)                     # [B,S,H]
    pre    = shared @ adapt_w.T                   # [B,S,A]
    for i in 0..7:
        h  = LN(pre @ adapter_w[i].T) * g[i] + b[i]
        o  = (h @ expert_proj_w.T) @ output_proj_w.T
        combined = where(mask_i, o, combined)     # overwrite: later experts win
    out = shared + 0.1 * combined

Algebraic restructure: the mask-overwrite selects, per token, the LAST
expert i with expert_weights > 0 (or zero if none).  expert_proj/
output_proj are shared across experts and linear, so selection commutes
with them; LN is per-token so selection commutes with LN too.  Per token
only ONE expert's tiny A->A adapter matmul is needed:
    sel  = sum_i onehot_i * (pre @ adapter_w[i].T)   # one-hot from host
    g    = LN(sel) * (0.1*gamma[e(t)]) + (0.1*beta[e(t)])   # 0 if unrouted
    out  = shared + (g @ expert_proj_w.T) @ output_proj_w.T
This removes the 8x duplication of the big GEMMs: ~53 GFLOP total.

Sharding: data-parallel over the 2048 tokens, 256 per core, weights
replicated (streamed from HBM once per core).  Device layout is
feature-major ("transposed", [feature, token]) except the adapter/LN
stage which is token-major so LN reduces along the free axis.
"""

import sys

if "/opt/trn_rl_repo" not in sys.path:
    sys.path.insert(0, "/opt/trn_rl_repo")

import numpy as np
import ml_dtypes

import concourse.bass as bass
import concourse.bacc as bacc
import concourse.mybir as mybir
import concourse.tile as tile
from concourse.masks import make_identity
from concourse.vector_clock import ScopedClock
from concourse.bass_utils import run_bass_kernel_spmd

BF16 = ml_dtypes.bfloat16

B, S, E, NE = 2, 1024, 768, 8
H = 4 * E            # 3072
A = H // 16          # 192
LN_EPS = 1e-5
N_CORES = 8
T = (B * S) // N_CORES   # 256 tokens per core
P = 128

KE = E // P          # 6  k-tiles over E
MH = H // P          # 24 m-tiles over H
KH = H // P          # 24 k-tiles over H
TH = T // P          # 2  token halves

# Matmul dtypes (tunable): float32r streams 1 row/cycle for free-dim>=256
# (same as bf16) while keeping fp32 storage.  The expert path is ~1e-5 of
# the output magnitude, so bf16 there is far below the noise floor.
DT_A = mybir.dt.float32r     # up-proj (dominates output accuracy)
DT_REST = mybir.dt.bfloat16  # adapt/adapter (tiny expert path)
DT_PROJ = mybir.dt.float8e4  # expert_proj/output_proj (big weight streams)
F32 = mybir.dt.float32

# power-of-2 scale folds so the tiny expert-path values fit fp8e4m3
G_S = 2.0 ** 13    # folded into gamma/beta on host (with the 0.1)
EP_S = 2.0 ** 10   # expert_proj weight scale (host)
OP_S = 2.0 ** 10   # output_proj weight scale (host)
F_EVICT_S = 2.0 ** -7     # t_act = F_psum * F_EVICT_S  (fp8-friendly ~1.0)
G_EVICT_S = 2.0 ** -26    # undoes OP_S * EP_S * G_S * F_EVICT_S

NP_DT = {mybir.dt.float32r: np.float32,
         mybir.dt.float32: np.float32,
         mybir.dt.float8e4: mybir.dt.np(mybir.dt.float8e4),
         mybir.dt.bfloat16: BF16}


class SplitDrainTileContext(tile.TileContext):
    """TileContext whose exit drain splits its sem waits one-per-Drain:
    walrus's CTRL lowering in this container accepts only a single sync
    wait on a Drain instruction."""

    def _drain_and_barrier(self, tick_clock, wait_clock):
        nc = self.nc
        drain_inst = nc.sync.drain()
        wait_clock.add_sem_waits(
            drain_inst.ins, ScopedClock({None: tick_clock.global_clock})
        )
        si = drain_inst.ins.sync_info
        waits = list(si.on_wait) if si is not None else []
        if len(waits) > 1:
            drain_inst.ins.sync_info = mybir.SyncInfo(
                on_wait=waits[:1], on_update=list(si.on_update)
            )
            for k in range(1, len(waits)):
                extra = nc.sync.drain()
                extra.ins.sync_info = mybir.SyncInfo(
                    on_wait=waits[k:k + 1], on_update=[]
                )
        nc.all_engine_barrier()
        popped = nc._tile_sem_poison_stack.pop()
        assert popped is self._sem_poison
        nc.clear_and_free_semaphores(list(self.sems.allocated().values()))
        nc.all_engine_barrier()


def _build_program():
    nc = bacc.Bacc()

    # ---- DRAM I/O (per core) ----
    xT = nc.dram_tensor("xT", [E, T], DT_A, kind="ExternalInput")
    up_wT = nc.dram_tensor("up_wT", [E, H], DT_A, kind="ExternalInput")
    adapt_wT = nc.dram_tensor("adapt_wT", [H, A], DT_REST, kind="ExternalInput")
    awTcat = nc.dram_tensor("awTcat", [2 * P, NE * A], DT_REST, kind="ExternalInput")
    oh = nc.dram_tensor("oh", [T, NE], F32, kind="ExternalInput")
    gam = nc.dram_tensor("gam", [T, A], F32, kind="ExternalInput")
    bet = nc.dram_tensor("bet", [T, A], F32, kind="ExternalInput")
    ep_wT = nc.dram_tensor("ep_wT", [2 * P, H], DT_PROJ, kind="ExternalInput")
    op_wT = nc.dram_tensor("op_wT", [H, H], DT_PROJ, kind="ExternalInput")
    out = nc.dram_tensor("out", [H, T], F32, kind="ExternalOutput")

    with tile.TileContext(nc) as tc:
        with (
            tc.tile_pool(name="const", bufs=1) as const_pool,
            tc.tile_pool(name="xw", bufs=1) as x_pool,
            tc.tile_pool(name="upw", bufs=3) as up_pool,
            tc.tile_pool(name="shared", bufs=1) as shared_pool,
            tc.tile_pool(name="small_w", bufs=1) as smallw_pool,
            tc.tile_pool(name="adapter", bufs=1) as ad_pool,
            tc.tile_pool(name="tact", bufs=1) as tact_pool,
            tc.tile_pool(name="opw", bufs=10) as op_pool,
            tc.tile_pool(name="outs", bufs=4) as out_pool,
            tc.tile_pool(name="psbig", bufs=4, space="PSUM") as psA,
            tc.tile_pool(name="pssmall", bufs=4, space="PSUM") as psS,
        ):
            # ---------- load shared inputs ----------
            x_sb = x_pool.tile([P, KE, T], DT_A)
            nc.sync.dma_start(out=x_sb[:], in_=xT.rearrange("(k p) t -> p k t", p=P))

            adapt_sb = smallw_pool.tile([P, KH, A], DT_REST)
            nc.sync.dma_start(
                out=adapt_sb[:], in_=adapt_wT.rearrange("(k p) a -> p k a", p=P)
            )
            aw_sb = ad_pool.tile([P, 2, NE * A], DT_REST)
            nc.sync.dma_start(
                out=aw_sb[:], in_=awTcat.rearrange("(k p) n -> p k n", p=P)
            )
            ep_sb = smallw_pool.tile([P, 2, H], DT_PROJ)
            nc.sync.dma_start(
                out=ep_sb[:], in_=ep_wT.rearrange("(k p) h -> p k h", p=P)
            )
            oh_sb = const_pool.tile([P, TH, NE], F32)
            nc.sync.dma_start(out=oh_sb[:], in_=oh.rearrange("(n p) i -> p n i", p=P))
            gam_sb = const_pool.tile([P, TH, A], F32)
            nc.sync.dma_start(out=gam_sb[:], in_=gam.rearrange("(n p) c -> p n c", p=P))
            bet_sb = const_pool.tile([P, TH, A], F32)
            nc.sync.dma_start(out=bet_sb[:], in_=bet.rearrange("(n p) c -> p n c", p=P))
            ident = const_pool.tile([P, P], F32)
            make_identity(nc, ident)
            eps_tile = const_pool.tile([P, 1], F32)
            nc.any.memset(eps_tile[:], float(LN_EPS))

            up_r = up_wT.rearrange("(k p) h -> p k h", p=P)
            op_r = op_wT.rearrange("(k p) g -> p k g", p=P)

            # ---------- Stage A: sharedT = silu(up_wT.T @ xT) ----------
            shared_f32 = []   # 24 tiles [128, T] fp32 (residual + accuracy)
            shared_mm = []    # bf16 copies for stage B rhs
            for m in range(MH):
                up_strip = up_pool.tile([P, KE, P], DT_A, tag="up_strip")
                nc.sync.dma_start(
                    out=up_strip[:], in_=up_r[:, :, m * P:(m + 1) * P]
                )
                ps = psA.tile([P, T], F32, tag="ps")
                for k in range(KE):
                    nc.tensor.matmul(
                        ps[:], up_strip[:, k, :], x_sb[:, k, :],
                        start=(k == 0), stop=(k == KE - 1),
                    )
                sh = shared_pool.tile([P, T], F32, tag=f"sh{m}")
                nc.scalar.activation(sh[:], ps[:], mybir.ActivationFunctionType.Silu)
                shb = shared_pool.tile([P, T], DT_REST, tag=f"shb{m}")
                nc.vector.tensor_copy(shb[:], sh[:])
                shared_f32.append(sh)
                shared_mm.append(shb)

            # ---------- Stage B: preT = adapt_wT.T @ sharedT  [A, T] ----------
            # A=192 -> two partition groups (128 + 64)
            pre_mm = []   # [ [128,T], [64,T] ] in DT_REST
            for g in range(2):
                gp = P if g == 0 else A - P      # 128, 64
                ps = psA.tile([P, T], F32, tag="ps")
                for k in range(KH):
                    nc.tensor.matmul(
                        ps[:gp, :],
                        adapt_sb[:, k, g * P:g * P + gp],
                        shared_mm[k][:],
                        start=(k == 0), stop=(k == KH - 1),
                    )
                pb = shared_pool.tile([P, T], DT_REST, tag=f"pre{g}")
                nc.scalar.activation(
                    pb[:gp, :], ps[:gp, :], mybir.ActivationFunctionType.Copy
                )
                pre_mm.append(pb)

            # ---------- Stage D/E: adapter + select + LayerNorm (token-major) ----------
            # gT: feature-major normalized output [A, T] in DT_REST
            gT0 = shared_pool.tile([P, T], DT_PROJ, tag="gT0")   # c 0:128
            gT1 = shared_pool.tile([P, T], DT_PROJ, tag="gT1")   # c 128:192 (64 rows)
            for th in range(TH):
                tsl = slice(th * P, (th + 1) * P)
                hsel = shared_pool.tile([P, A], F32, tag="hsel")
                for i in range(NE):
                    ps = psS.tile([P, A], F32, tag="pss")
                    nc.tensor.matmul(
                        ps[:], pre_mm[0][:, tsl], aw_sb[:, 0, i * A:(i + 1) * A],
                        start=True, stop=False,
                    )
                    nc.tensor.matmul(
                        ps[:], pre_mm[1][:64, tsl], aw_sb[:64, 1, i * A:(i + 1) * A],
                        start=False, stop=True,
                    )
                    if i == 0:
                        nc.scalar.activation(
                            hsel[:], ps[:], mybir.ActivationFunctionType.Copy,
                            scale=oh_sb[:, th, i:i + 1],
                        )
                    else:
                        hm = shared_pool.tile([P, A], F32, tag="hm")
                        nc.scalar.activation(
                            hm[:], ps[:], mybir.ActivationFunctionType.Copy,
                            scale=oh_sb[:, th, i:i + 1],
                        )
                        nc.vector.tensor_add(hsel[:], hsel[:], hm[:])

                # LayerNorm over free axis (192), biased var, eps inside sqrt
                s1 = shared_pool.tile([P, 1], F32, tag="s1")
                nc.vector.reduce_sum(s1[:], hsel[:], axis=mybir.AxisListType.X)
                hsq = shared_pool.tile([P, A], F32, tag="hsq")
                s2 = shared_pool.tile([P, 1], F32, tag="s2")
                nc.scalar.activation(
                    hsq[:], hsel[:], mybir.ActivationFunctionType.Square,
                    accum_out=s2[:],
                )
                t1 = shared_pool.tile([P, 1], F32, tag="t1")
                nc.vector.tensor_mul(t1[:], s1[:], s1[:])
                nc.vector.tensor_scalar_mul(t1[:], t1[:], 1.0 / A)
                nc.vector.tensor_sub(t1[:], s2[:], t1[:])        # 192*var
                std = shared_pool.tile([P, 1], F32, tag="std")
                nc.scalar.activation(
                    std[:], t1[:], mybir.ActivationFunctionType.Sqrt,
                    scale=1.0 / A, bias=eps_tile[:],
                )
                rstd = shared_pool.tile([P, 1], F32, tag="rstd")
                nc.vector.reciprocal(rstd[:], std[:])
                nm = shared_pool.tile([P, 1], F32, tag="nm")
                nc.vector.tensor_mul(nm[:], s1[:], rstd[:])
                nc.vector.tensor_scalar_mul(nm[:], nm[:], -1.0 / A)
                gtok = shared_pool.tile([P, A], F32, tag="gtok")
                nc.scalar.activation(
                    gtok[:], hsel[:], mybir.ActivationFunctionType.Identity,
                    scale=rstd[:], bias=nm[:],
                )
                nc.vector.tensor_mul(gtok[:], gtok[:], gam_sb[:, th, :])
                nc.vector.tensor_add(gtok[:], gtok[:], bet_sb[:, th, :])

                # transpose to feature-major: [128t, 192c] -> [c, 128t]
                pst = psS.tile([P, P], F32, tag="pss")
                nc.tensor.transpose(pst[:], gtok[:, 0:P], ident[:])
                nc.scalar.activation(
                    gT0[:, tsl], pst[:], mybir.ActivationFunctionType.Copy
                )
                pst2 = psS.tile([P, P], F32, tag="pss")
                nc.tensor.transpose(pst2[:64, :], gtok[:, P:A], ident[:])
                nc.scalar.activation(
                    gT1[:64, tsl], pst2[:64, :], mybir.ActivationFunctionType.Copy
                )

            # ---------- Stage F: t_actT = ep_wT.T @ gT  [H, T] ----------
            tact_mm = []
            for m in range(MH):
                ps = psA.tile([P, T], F32, tag="ps")
                nc.tensor.matmul(
                    ps[:], ep_sb[:, 0, m * P:(m + 1) * P], gT0[:],
                    start=True, stop=False,
                )
                nc.tensor.matmul(
                    ps[:], ep_sb[:64, 1, m * P:(m + 1) * P], gT1[:64, :],
                    start=False, stop=True,
                )
                ta = tact_pool.tile([P, T], DT_PROJ, tag=f"ta{m}")
                nc.scalar.activation(
                    ta[:], ps[:], mybir.ActivationFunctionType.Copy,
                    scale=float(F_EVICT_S),
                )
                tact_mm.append(ta)

            # ---------- Stage G: out = sharedT + op_wT.T @ t_actT ----------
            for m in range(MH):
                op_strip = op_pool.tile([P, KH, P], DT_PROJ, tag="op_strip")
                nc.sync.dma_start(
                    out=op_strip[:], in_=op_r[:, :, m * P:(m + 1) * P]
                )
                ps = psA.tile([P, T], F32, tag="ps")
                for k in range(KH):
                    nc.tensor.matmul(
                        ps[:], op_strip[:, k, :], tact_mm[k][:],
                        start=(k == 0), stop=(k == KH - 1),
                    )
                og = out_pool.tile([P, T], F32, tag="og")
                nc.scalar.activation(
                    og[:], ps[:], mybir.ActivationFunctionType.Copy,
                    scale=float(G_EVICT_S),
                )
                ot = out_pool.tile([P, T], F32, tag="ot")
                nc.vector.tensor_add(ot[:], og[:], shared_f32[m][:])
                nc.sync.dma_start(out=out[m * P:(m + 1) * P, :], in_=ot[:])

    nc.finalize()
    return nc


_NC_CACHE = None
LAST_RUN_S = None  # wall time of the last device dispatch (incl. RPC)


def _get_program():
    global _NC_CACHE
    if _NC_CACHE is None:
        _NC_CACHE = _build_program()
    return _NC_CACHE


def kernel(x, expert_weights, up_w, adapt_w, adapter_w, ln_gamma, ln_beta,
           expert_proj_w, output_proj_w):
    x = np.asarray(x, dtype=np.float32)
    expert_weights = np.asarray(expert_weights, dtype=np.float32)
    up_w = np.asarray(up_w, dtype=np.float32)
    adapt_w = np.asarray(adapt_w, dtype=np.float32)
    adapter_w = np.asarray(adapter_w, dtype=np.float32)
    ln_gamma = np.asarray(ln_gamma, dtype=np.float32)
    ln_beta = np.asarray(ln_beta, dtype=np.float32)
    expert_proj_w = np.asarray(expert_proj_w, dtype=np.float32)
    output_proj_w = np.asarray(output_proj_w, dtype=np.float32)

    NT = B * S  # 2048
    rest_np = NP_DT[DT_REST]

    # ---- routing (host): last expert with weight > 0, one-hot ----
    ew = expert_weights.reshape(NT, NE)
    pos = ew > 0
    idx = (NE - 1) - pos[:, ::-1].argmax(axis=1)       # last True (0 if none)
    valid = pos.any(axis=1)
    idx = np.where(valid, idx, 0)
    oh_full = np.zeros((NT, NE), np.float32)
    oh_full[np.arange(NT), idx] = valid.astype(np.float32)
    # fold the 0.1 output scale + unrouted-token zeroing into gamma/beta
    vmask = valid.astype(np.float32)[:, None]
    gam_full = (ln_gamma[idx] * (0.1 * G_S) * vmask).astype(np.float32)
    bet_full = (ln_beta[idx] * (0.1 * G_S) * vmask).astype(np.float32)

    # ---- weight prep (host, replicated across cores) ----
    xT_full = np.ascontiguousarray(x.reshape(NT, E).T)            # [E, NT] f32
    up_wT = np.ascontiguousarray(up_w.T)                          # [E, H] f32
    adapt_wT = np.ascontiguousarray(adapt_w.T).astype(rest_np)    # [H, A]
    awTcat = np.zeros((2 * P, NE * A), rest_np)                   # [256, 1536]
    awT = adapter_w.transpose(0, 2, 1)                            # [NE, A(in), A(out)]
    awTcat[:A, :] = np.ascontiguousarray(
        awT.transpose(1, 0, 2).reshape(A, NE * A)
    ).astype(rest_np)
    proj_np = NP_DT[DT_PROJ]
    ep_wTp = np.zeros((2 * P, H), proj_np)
    ep_wTp[:A, :] = (expert_proj_w.T * EP_S).astype(proj_np)      # [A, H] padded
    op_wT = np.ascontiguousarray(
        (output_proj_w.T * OP_S).astype(proj_np))                 # [H, H]

    in_maps = []
    for c in range(N_CORES):
        tsl = slice(c * T, (c + 1) * T)
        in_maps.append({
            "xT": np.ascontiguousarray(xT_full[:, tsl]),
            "up_wT": up_wT,
            "adapt_wT": adapt_wT,
            "awTcat": awTcat,
            "oh": np.ascontiguousarray(oh_full[tsl]),
            "gam": np.ascontiguousarray(gam_full[tsl]),
            "bet": np.ascontiguousarray(bet_full[tsl]),
            "ep_wT": ep_wTp,
            "op_wT": op_wT,
        })

    import time
    nc = _get_program()
    t0 = time.perf_counter()
    res = run_bass_kernel_spmd(nc, in_maps, list(range(N_CORES)))
    global LAST_RUN_S
    LAST_RUN_S = time.perf_counter() - t0

    outs = [res.results[c]["out"].T for c in range(N_CORES)]      # [T, H] each
    full = np.concatenate(outs, axis=0)                           # [NT, H]
    return np.ascontiguousarray(full.reshape(B, S, H)).astype(np.float32)


# revision 23
# speedup vs baseline: 1.8516x; 1.4595x over previous
"""Trainium2 Bass kernel for nn_ExpertGroup (moe_routing).

Reference computation (B=2, S=1024, E=768, NE=8, H=3072, A=192):
    shared = silu(x @ up_w.T)                     # [B,S,H]
    pre    = shared @ adapt_w.T                   # [B,S,A]
    for i in 0..7:
        h  = LN(pre @ adapter_w[i].T) * g[i] + b[i]
        o  = (h @ expert_proj_w.T) @ output_proj_w.T
        combined = where(mask_i, o, combined)     # overwrite: later experts win
    out = shared + 0.1 * combined

Algebraic restructure: the mask-overwrite selects, per token, the LAST
expert i with expert_weights > 0 (or zero if none).  expert_proj/
output_proj are shared across experts and linear, so selection commutes
with them; LN is per-token so selection commutes with LN too.  Per token
only ONE expert's tiny A->A adapter matmul is needed:
    sel  = sum_i onehot_i * (pre @ adapter_w[i].T)   # one-hot from host
    g    = LN(sel) * (0.1*gamma[e(t)]) + (0.1*beta[e(t)])   # 0 if unrouted
    out  = shared + (g @ expert_proj_w.T) @ output_proj_w.T
This removes the 8x duplication of the big GEMMs: ~53 GFLOP total.

Sharding: data-parallel over the 2048 tokens, 256 per core, weights
replicated (streamed from HBM once per core).

Precision: the expert path contributes ~1.3e-5 of the output magnitude
(LN_EPS dominates the tiny adapter variance), so everything downstream
of `pre` runs in fp8e4m3 with power-of-2 scale folds and DoubleRow
matmuls (2 fp8 MACs/cell/cycle).  The accuracy-critical up-projection
runs in fp16 (10 mantissa bits) with fp32 PSUM accumulation.
"""

import sys

if "/opt/trn_rl_repo" not in sys.path:
    sys.path.insert(0, "/opt/trn_rl_repo")

import numpy as np
import ml_dtypes

import concourse.bass as bass
import concourse.bacc as bacc
import concourse.mybir as mybir
import concourse.tile as tile
from concourse.masks import make_identity
from concourse.bass_utils import run_bass_kernel_spmd

BF16 = ml_dtypes.bfloat16

B, S, E, NE = 2, 1024, 768, 8
H = 4 * E            # 3072
A = H // 16          # 192
LN_EPS = 1e-5
N_CORES = 8
T = (B * S) // N_CORES   # 256 tokens per core
P = 128

KE = E // P          # 6  k-tiles over E
MH = H // P          # 24 m/k-tiles over H
KH = H // P
TH = T // P          # 2  token halves

DT_A = mybir.dt.float16      # up-proj (dominates output accuracy)
DT_8 = mybir.dt.float8e4     # everything downstream of `pre`
F32 = mybir.dt.float32
DR = mybir.MatmulPerfMode.DoubleRow

# power-of-2 scale folds so the tiny expert-path values use fp8e4m3's range
AD_S = 2.0 ** 8           # adapt_w weight scale (host)
B_EVICT_S = 2.0 ** -1     # preT = B_psum * this = 2^7 * pre
AW_S = 2.0 ** 10          # adapter_w weight scale (host)
OH_S = 2.0 ** -17         # folded into the host one-hot (undoes 2^7 * 2^10)
G_S = 2.0 ** 13           # folded into gamma/beta on host (with the 0.1)
EP_S = 2.0 ** 10          # expert_proj weight scale (host)
OP_S = 2.0 ** 10          # output_proj weight scale (host)
F_EVICT_S = 2.0 ** -7     # t_act = F_psum * this  (fp8-friendly ~1.0 std)
G_EVICT_S = 2.0 ** -26    # undoes OP_S * (EP_S * G_S * F_EVICT_S)

NP_DT = {mybir.dt.float16: np.float16,
         mybir.dt.float32: np.float32,
         mybir.dt.float8e4: mybir.dt.np(mybir.dt.float8e4),
         mybir.dt.bfloat16: BF16}


def _build_program():
    nc = bacc.Bacc()

    # ---- DRAM I/O (per core) ----
    xT = nc.dram_tensor("xT", [P, KE, T], DT_A, kind="ExternalInput")
    up_wT = nc.dram_tensor("up_wT", [4, P, MH // 4, KE, P], DT_A, kind="ExternalInput")
    adapt_wT = nc.dram_tensor("adapt_wT", [P, KH, A], DT_8, kind="ExternalInput")
    awTcat = nc.dram_tensor("awTcat", [A, NE * A], DT_8, kind="ExternalInput")
    oh = nc.dram_tensor("oh", [T, NE], F32, kind="ExternalInput")
    gam = nc.dram_tensor("gam", [T, A], mybir.dt.bfloat16, kind="ExternalInput")
    bet = nc.dram_tensor("bet", [T, A], mybir.dt.bfloat16, kind="ExternalInput")
    ep_wT = nc.dram_tensor("ep_wT", [A, H], DT_8, kind="ExternalInput")
    op_wT = nc.dram_tensor("op_wT", [6, P, MH // 6, KH, P], DT_8, kind="ExternalInput")
    out = nc.dram_tensor("out", [H, T], mybir.dt.float16, kind="ExternalOutput")

    with tile.TileContext(nc) as tc:
        with (
            tc.tile_pool(name="const", bufs=1) as const_pool,
            tc.tile_pool(name="xw", bufs=1) as x_pool,
            tc.tile_pool(name="upw", bufs=2) as up_pool,
            tc.tile_pool(name="shared", bufs=1) as shared_pool,
            tc.tile_pool(name="small_w", bufs=1) as smallw_pool,
            tc.tile_pool(name="adapter", bufs=1) as ad_pool,
            tc.tile_pool(name="tact", bufs=1) as tact_pool,
            tc.tile_pool(name="opw", bufs=4) as op_pool,
            tc.tile_pool(name="outs", bufs=4) as out_pool,
            tc.tile_pool(name="psbig", bufs=4, space="PSUM") as psA,
            tc.tile_pool(name="pssmall", bufs=4, space="PSUM") as psS,
        ):
            # ---------- x load (stage A needs it first) ----------
            x_sb = x_pool.tile([P, KE, T], DT_A)
            nc.sync.dma_start(out=x_sb[:], in_=xT[:])

            small = {}

            def _load_small_weights():
                t_ = smallw_pool.tile([P, KH, A], DT_8, tag="adapt_sb", name="adapt_sb")
                nc.sync.dma_start(out=t_[:], in_=adapt_wT[:])
                small["adapt_sb"] = t_
                t_ = ad_pool.tile([P, 2, NE * A], DT_8, tag="aw_sb", name="aw_sb")
                nc.sync.dma_start(out=t_[:, 0, :], in_=awTcat[0:P, :])
                nc.sync.dma_start(out=t_[0:A - P, 1, :], in_=awTcat[P:A, :])
                nc.any.memset(t_[A - P:P, 1, :], 0.0)
                small["aw_sb"] = t_
                t_ = smallw_pool.tile([P, 2, H], DT_8, tag="ep_sb", name="ep_sb")
                nc.sync.dma_start(out=t_[:, 0, :], in_=ep_wT[0:P, :])
                nc.sync.dma_start(out=t_[0:A - P, 1, :], in_=ep_wT[P:A, :])
                nc.any.memset(t_[A - P:P, 1, :], 0.0)
                small["ep_sb"] = t_
                t_ = const_pool.tile([P, TH, NE], F32, tag="oh_sb", name="oh_sb")
                nc.sync.dma_start(out=t_[:], in_=oh.rearrange("(n p) i -> p n i", p=P))
                small["oh_sb"] = t_
                t_ = const_pool.tile([P, TH, A], mybir.dt.bfloat16, tag="gam_sb", name="gam_sb")
                nc.sync.dma_start(out=t_[:], in_=gam.rearrange("(n p) c -> p n c", p=P))
                small["gam_sb"] = t_
                t_ = const_pool.tile([P, TH, A], mybir.dt.bfloat16, tag="bet_sb", name="bet_sb")
                nc.sync.dma_start(out=t_[:], in_=bet.rearrange("(n p) c -> p n c", p=P))
                small["bet_sb"] = t_
                t_ = const_pool.tile([P, P], F32, tag="ident", name="ident")
                make_identity(nc, t_)
                small["ident"] = t_
                t_ = const_pool.tile([P, 1], F32, tag="eps", name="eps")
                nc.any.memset(t_[:], float(LN_EPS))
                small["eps_tile"] = t_

            # ---------- Stage A: sharedT = silu(up_wT.T @ xT) ----------
            shared_f32 = []     # 24 tiles [128, T] fp32 (residual)
            sh_pair = [shared_pool.tile([P, 2, T], DT_8, tag=f"shp{j}", name=f"shp{j}")
                       for j in range(MH // 2)]   # fp8 pairs: B's DoubleRow rhs
            GA = MH // 4      # 6 strips per up group
            for g in range(4):
                up_grp = up_pool.tile([P, GA, KE, P], DT_A, tag="up_grp")
                nc.sync.dma_start(out=up_grp[:], in_=up_wT[g])
                for s in range(GA):
                    m = g * GA + s
                    ps = psA.tile([P, T], F32, tag="ps")
                    for k in range(KE):
                        nc.tensor.matmul(
                            ps[:], up_grp[:, s, k, :], x_sb[:, k, :],
                            start=(k == 0), stop=(k == KE - 1),
                        )
                    sh = shared_pool.tile([P, T], F32, tag=f"sh{m}")
                    nc.scalar.activation(
                        sh[:], ps[:], mybir.ActivationFunctionType.Silu
                    )
                    nc.vector.tensor_copy(sh_pair[m // 2][:, m % 2, :], sh[:])
                    shared_f32.append(sh)

            _load_small_weights()
            adapt_sb = small["adapt_sb"]; aw_sb = small["aw_sb"]
            ep_sb = small["ep_sb"]; oh_sb = small["oh_sb"]
            gam_sb = small["gam_sb"]; bet_sb = small["bet_sb"]
            ident = small["ident"]; eps_tile = small["eps_tile"]

            # ---------- Stage B: preT = (2^8 adapt_wT).T @ sharedT ----------
            # A=192 -> two partition groups (128 + 64); DoubleRow over k-pairs
            pre3 = shared_pool.tile([P, 2, T], DT_8, tag="pre3")
            # zero the pad rows (fp8 garbage could be NaN; 0*NaN poisons PSUM)
            nc.any.memset(pre3[64:P, 1, :], 0.0)
            for g in range(2):
                gp = P if g == 0 else A - P      # 128, 64
                ps = psA.tile([P, T], F32, tag="ps")
                for j in range(MH // 2):
                    nc.tensor.matmul(
                        ps[:gp, :],
                        adapt_sb[:, 2 * j:2 * j + 2, g * P:g * P + gp],
                        sh_pair[j][:],
                        start=(j == 0), stop=(j == MH // 2 - 1),
                        perf_mode=DR,
                    )
                nc.vector.tensor_scalar_mul(
                    pre3[:gp, g, :], ps[:gp, :], float(B_EVICT_S)
                )

            # ---------- Stage D/E: adapter + select + LayerNorm (token-major) ----------
            gT3 = shared_pool.tile([P, 2, T], DT_8, tag="gT3")  # F's DoubleRow rhs
            nc.any.memset(gT3[64:P, 1, :], 0.0)
            for th in range(TH):
                tsl = slice(th * P, (th + 1) * P)
                hsel = shared_pool.tile([P, A], F32, tag="hsel")
                hparts = []
                for i in range(NE):
                    ps = psS.tile([P, A], F32, tag="pss")
                    nc.tensor.matmul(
                        ps[:], pre3[:, :, tsl], aw_sb[:, :, i * A:(i + 1) * A],
                        start=True, stop=True, perf_mode=DR,
                    )
                    hm = shared_pool.tile([P, A], F32, tag=f"hm{i}", name=f"hm{i}")
                    nc.vector.tensor_scalar_mul(
                        hm[:], ps[:], oh_sb[:, th, i:i + 1]
                    )
                    hparts.append(hm)
                # pairwise reduction tree keeps the DVE chain short
                lvl = 0
                while len(hparts) > 1:
                    nxt = []
                    for j in range(0, len(hparts), 2):
                        if len(hparts) > 2:
                            dst = shared_pool.tile(
                                [P, A], F32, tag=f"ht{lvl}_{j}", name=f"ht{lvl}_{j}"
                            )
                        else:
                            dst = hsel
                        nc.vector.tensor_add(dst[:], hparts[j][:], hparts[j + 1][:])
                        nxt.append(dst)
                    hparts = nxt
                    lvl += 1

                # LayerNorm over free axis (192), biased var, eps inside sqrt
                s1 = shared_pool.tile([P, 1], F32, tag="s1")
                nc.vector.reduce_sum(s1[:], hsel[:], axis=mybir.AxisListType.X)
                hsq = shared_pool.tile([P, A], F32, tag="hsq")
                s2 = shared_pool.tile([P, 1], F32, tag="s2")
                nc.scalar.activation(
                    hsq[:], hsel[:], mybir.ActivationFunctionType.Square,
                    accum_out=s2[:],
                )
                t1 = shared_pool.tile([P, 1], F32, tag="t1")
                nc.vector.tensor_mul(t1[:], s1[:], s1[:])
                nc.vector.tensor_scalar_mul(t1[:], t1[:], 1.0 / A)
                nc.vector.tensor_sub(t1[:], s2[:], t1[:])        # 192*var
                std = shared_pool.tile([P, 1], F32, tag="std")
                nc.scalar.activation(
                    std[:], t1[:], mybir.ActivationFunctionType.Sqrt,
                    scale=1.0 / A, bias=eps_tile[:],
                )
                rstd = shared_pool.tile([P, 1], F32, tag="rstd")
                nc.vector.reciprocal(rstd[:], std[:])
                nm = shared_pool.tile([P, 1], F32, tag="nm")
                nc.vector.tensor_mul(nm[:], s1[:], rstd[:])
                nc.vector.tensor_scalar_mul(nm[:], nm[:], -1.0 / A)
                gtok = shared_pool.tile([P, A], F32, tag="gtok")
                nc.vector.tensor_scalar(
                    gtok[:], hsel[:], rstd[:], nm[:],
                    mybir.AluOpType.mult, mybir.AluOpType.add,
                )
                nc.vector.tensor_mul(gtok[:], gtok[:], gam_sb[:, th, :])
                nc.vector.tensor_add(gtok[:], gtok[:], bet_sb[:, th, :])

                # transpose to feature-major fp8 pairs: [128t, 192c] -> gT3
                pst = psS.tile([P, P], F32, tag="pss")
                nc.tensor.transpose(pst[:], gtok[:, 0:P], ident[:])
                nc.vector.tensor_copy(gT3[:, 0, tsl], pst[:])
                pst2 = psS.tile([P, P], F32, tag="pss")
                nc.tensor.transpose(pst2[:64, :], gtok[:, P:A], ident[:])
                nc.vector.tensor_copy(gT3[:64, 1, tsl], pst2[:64, :])

            # ---------- Stage F: t_actT = (2^10 ep_wT).T @ gT ----------
            ta_pair = [tact_pool.tile([P, 2, T], DT_8, tag=f"tap{j}", name=f"tap{j}")
                       for j in range(MH // 2)]   # fp8 pairs: G's DoubleRow rhs
            for m in range(MH):
                ps = psA.tile([P, T], F32, tag="ps")
                nc.tensor.matmul(
                    ps[:], ep_sb[:, :, m * P:(m + 1) * P], gT3[:],
                    start=True, stop=True, perf_mode=DR,
                )
                nc.vector.tensor_scalar_mul(
                    ta_pair[m // 2][:, m % 2, :], ps[:], float(F_EVICT_S)
                )

            # ---------- Stage G: out = sharedT + (2^10 op_wT).T @ t_actT ----------
            out_r = out.rearrange("(mm p) t -> p mm t", p=P)
            GO = MH // 6      # 4 strips per op group
            for g in range(6):
                op_grp = op_pool.tile([P, GO, KH, P], DT_8, tag="op_grp")
                nc.sync.dma_start(out=op_grp[:], in_=op_wT[g])
                ot = out_pool.tile([P, GO, T], mybir.dt.float16, tag="ot")
                for s in range(GO):
                    m = g * GO + s
                    ps = psA.tile([P, T], F32, tag="ps")
                    for j in range(MH // 2):
                        nc.tensor.matmul(
                            ps[:], op_grp[:, s, 2 * j:2 * j + 2, :], ta_pair[j][:],
                            start=(j == 0), stop=(j == MH // 2 - 1),
                            perf_mode=DR,
                        )
                    og = out_pool.tile([P, T], F32, tag="og")
                    nc.vector.tensor_scalar_mul(og[:], ps[:], float(G_EVICT_S))
                    nc.vector.tensor_add(ot[:, s, :], og[:], shared_f32[m][:])
                nc.sync.dma_start(out=out_r[:, g * GO:(g + 1) * GO, :], in_=ot[:])

    nc.finalize()
    return nc


_NC_CACHE = None
LAST_RUN_S = None  # wall time of the last device dispatch (incl. RPC)


def _get_program():
    global _NC_CACHE
    if _NC_CACHE is None:
        _NC_CACHE = _build_program()
    return _NC_CACHE


def kernel(x, expert_weights, up_w, adapt_w, adapter_w, ln_gamma, ln_beta,
           expert_proj_w, output_proj_w):
    x = np.asarray(x, dtype=np.float32)
    expert_weights = np.asarray(expert_weights, dtype=np.float32)
    up_w = np.asarray(up_w, dtype=np.float32)
    adapt_w = np.asarray(adapt_w, dtype=np.float32)
    adapter_w = np.asarray(adapter_w, dtype=np.float32)
    ln_gamma = np.asarray(ln_gamma, dtype=np.float32)
    ln_beta = np.asarray(ln_beta, dtype=np.float32)
    expert_proj_w = np.asarray(expert_proj_w, dtype=np.float32)
    output_proj_w = np.asarray(output_proj_w, dtype=np.float32)

    NT = B * S  # 2048

    # ---- routing (host): last expert with weight > 0, one-hot ----
    ew = expert_weights.reshape(NT, NE)
    pos = ew > 0
    idx = (NE - 1) - pos[:, ::-1].argmax(axis=1)       # last True (0 if none)
    valid = pos.any(axis=1)
    idx = np.where(valid, idx, 0)
    oh_full = np.zeros((NT, NE), np.float32)
    oh_full[np.arange(NT), idx] = valid.astype(np.float32) * OH_S
    # fold the 0.1 output scale + unrouted-token zeroing into gamma/beta
    vmask = valid.astype(np.float32)[:, None]
    gam_full = (ln_gamma[idx] * (0.1 * G_S) * vmask).astype(BF16)
    bet_full = (ln_beta[idx] * (0.1 * G_S) * vmask).astype(BF16)

    # ---- weight prep (host, replicated across cores) ----
    a_np = NP_DT[DT_A]
    f8 = NP_DT[DT_8]
    # strip-major prepacked layouts: DMA reads become fully contiguous
    xT_full = np.ascontiguousarray(
        x.reshape(NT, E).T.reshape(KE, P, NT).transpose(1, 0, 2)
    ).astype(a_np)                                                    # [P, KE, NT]
    up_wT = np.ascontiguousarray(
        up_w.T.reshape(KE, P, MH, P).transpose(2, 1, 0, 3)            # [m, p, k, c]
        .reshape(4, MH // 4, P, KE, P).transpose(0, 2, 1, 3, 4)       # [g, p, s, k, c]
    ).astype(a_np)
    adapt_wT = np.ascontiguousarray(
        (adapt_w.T * AD_S).reshape(KH, P, A).transpose(1, 0, 2)
    ).astype(f8)                                                      # [P, KH, A]
    awT = adapter_w.transpose(0, 2, 1)          # [NE, A(in), A(out)]
    awTcat = (np.ascontiguousarray(
        awT.transpose(1, 0, 2).reshape(A, NE * A)) * AW_S).astype(f8)
    ep_wTp = np.ascontiguousarray((expert_proj_w.T * EP_S).astype(f8))  # [A, H]
    op_wT = np.ascontiguousarray(
        (output_proj_w.T * OP_S)
        .reshape(KH, P, MH, P).transpose(2, 1, 0, 3)                  # [m, p, k, c]
        .reshape(6, MH // 6, P, KH, P).transpose(0, 2, 1, 3, 4)       # [g, p, s, k, c]
    ).astype(f8)

    in_maps = []
    for c in range(N_CORES):
        tsl = slice(c * T, (c + 1) * T)
        in_maps.append({
            "xT": np.ascontiguousarray(xT_full[:, :, tsl]),
            "up_wT": up_wT,
            "adapt_wT": adapt_wT,
            "awTcat": awTcat,
            "oh": np.ascontiguousarray(oh_full[tsl]),
            "gam": np.ascontiguousarray(gam_full[tsl]),
            "bet": np.ascontiguousarray(bet_full[tsl]),
            "ep_wT": ep_wTp,
            "op_wT": op_wT,
        })

    import time
    nc = _get_program()
    t0 = time.perf_counter()
    res = run_bass_kernel_spmd(nc, in_maps, list(range(N_CORES)))
    global LAST_RUN_S
    LAST_RUN_S = time.perf_counter() - t0

    outs = [res.results[c]["out"].astype(np.float32).T for c in range(N_CORES)]      # [T, H] each
    full = np.concatenate(outs, axis=0)                           # [NT, H]
    return np.ascontiguousarray(full.reshape(B, S, H)).astype(np.float32)
